# revision 26
# baseline (speedup 1.0000x reference)
# Multi-headed self-attention (B=4, S=2048, D=1024, H=16) on 8 TRN2 NeuronCores.
#
# Sharding: tensor-parallel over heads. Core c computes heads 2c, 2c+1 (=128
# output columns) for all batches. Host pre-transposes x -> xT [D, B*S] and the
# per-core weight slices -> [D, 128] so every matmul contracts over the
# partition dimension. Host gathers the 8 [B*S, 128] outputs into (B,S,D).
#
# Per-core dataflow (fp32 storage, float32r matmul mode):
#   1. Projections: QT/KT/VT [128(2 heads x 64), 8192] = W.T-slices @ xT,
#      accumulated over 8 d-chunks in PSUM; bias added during the PSUM->SBUF
#      copy (DVE per-partition scalar add).
#   2. V2 prep: one packed PE transpose per 128-t chunk turns VT[128(2 heads),
#      128t] into [128t, 128w]; DVE applies the key mask and appends a mask
#      column per head -> v2 chunk layout [V_h0(64)|m|V_h1(64)|m] (130 cols).
#   3. Attention per (batch, q-block): scoresT tiles [128 k, 512 q] for BOTH
#      heads per k-chunk, head0 on PE row-tile (0,0) (partitions 0:64), head1
#      on row-tile (64,0) -> the two K=64 matmuls execute concurrently on
#      disjoint row groups. exp on alternating engines: ScalarE exact Exp
#      (fused 1/8 scale) for even k-chunks, VectorE Schraudolph bit-trick exp
#      (one tensor_scalar: int32(x*A+B) reinterpreted as f32, ~1.8% rel err,
#      softmax-safe) for odd k-chunks. No row-max subtraction (scores std
#      ~0.4, exp is safe in fp32, softmax is shift-invariant). PV matmuls use
#      the 65-col stationary [V|mask] so the accumulation yields unnormalized
#      h^T plus the softmax denominator. PE-transpose h''^T back to [q, 65],
#      DVE reciprocal of column 64, per-partition scalar multiply, DMA out.
#   The 0/1 mask is exact this way: reference's exp(-10000) == 0.0 in fp32.
#   Emission interleaves proj/v2-prep of batch b+1 with attention of batch b
#   to keep the PE dense; hts PSUM->SBUF copies run on GpSimd.

import sys

import numpy as np

B, S, D, H = 4, 2048, 1024, 16
NC = 8
HPC = H // NC  # heads per core = 2
WH = D // H  # head width = 64
CW = HPC * WH  # per-core output width = 128
BS = B * S  # 8192
DCH = D // 128  # d chunks = 8
QB = S // 512  # q blocks per batch = 4
KCH = S // 128  # k chunks per batch = 16
VCOLS = 2 * (WH + 1)  # v2 chunk cols = 130

# Schraudolph exp in bf16: exp(x*0.125) ~= bitcast_bf16(int16(x*A + Bc))
# (bf16 = 8-bit exp, 7-bit mantissa -> the int domain is 2^7 per octave)
_LN2 = float(np.log(2.0))
SCH_A = 0.125 * (2**7) / _LN2
SCH_B = 127.0 * (2**7) - 5.5
# k-chunks whose exp runs on VectorE (Schraudolph); rest on ScalarE (exact).
DVE_KCS = frozenset({1, 3, 5, 8, 10, 13, 15})

_CACHE = {}


def _ensure_import():
    try:
        import concourse.bass  # noqa: F401
    except ImportError:
        sys.path.insert(0, "/opt/trn_rl_repo")
        import concourse.bass  # noqa: F401


def build_bass():
    if "nc" in _CACHE:
        return _CACHE["nc"]
    _ensure_import()
    import concourse.mybir as mybir
    import concourse.tile as tile
    from concourse import bacc
    from concourse.masks import make_identity

    f32 = mybir.dt.float32
    f32r = mybir.dt.float32r
    bf16 = mybir.dt.bfloat16
    i16 = mybir.dt.int16
    AF = mybir.ActivationFunctionType
    ALU = mybir.AluOpType

    nc = bacc.Bacc(
        "TRN2",
        target_bir_lowering=False,
        debug=False,
        enable_asserts=False,
        num_devices=NC,
    )
    xT_d = nc.dram_tensor("xT", (D, BS), f32r, kind="ExternalInput").ap()
    wq_d = nc.dram_tensor("wqT", (D, CW), f32r, kind="ExternalInput").ap()
    wk_d = nc.dram_tensor("wkT", (D, CW), f32r, kind="ExternalInput").ap()
    wv_d = nc.dram_tensor("wvT", (D, CW), f32r, kind="ExternalInput").ap()
    bq_d = nc.dram_tensor("bq", (CW, 1), f32, kind="ExternalInput").ap()
    bk_d = nc.dram_tensor("bk", (CW, 1), f32, kind="ExternalInput").ap()
    bv_d = nc.dram_tensor("bv", (CW, 1), f32, kind="ExternalInput").ap()
    mask_d = nc.dram_tensor("maskT", (128, B * KCH), f32, kind="ExternalInput").ap()
    out_d = nc.dram_tensor("h_out", (BS, CW), f32, kind="ExternalOutput").ap()

    with tile.TileContext(nc) as tc:
        with (
            tc.tile_pool(name="qkv", bufs=1) as qkv_pool,
            tc.tile_pool(name="xt", bufs=20) as xt_pool,
            tc.tile_pool(name="wsb", bufs=1) as w_pool,
            tc.tile_pool(name="probs", bufs=4) as probs_pool,
            tc.tile_pool(name="v2", bufs=2) as v2_pool,
            tc.tile_pool(name="hts", bufs=4) as hts_pool,
            tc.tile_pool(name="ho", bufs=3) as ho_pool,
            tc.tile_pool(name="rc", bufs=8) as rc_pool,
            tc.tile_pool(name="cst", bufs=1) as cst_pool,
            tc.tile_pool(name="ps_sc", bufs=1, space="PSUM") as ps_sc,
            tc.tile_pool(name="ps_ht", bufs=4, space="PSUM") as ps_ht,
            tc.tile_pool(name="ps_acc", bufs=1, space="PSUM") as ps_acc,
            tc.tile_pool(name="ps_tr", bufs=1, space="PSUM") as ps_tr,
        ):
            ident = cst_pool.tile([128, 128], f32, tag="ident")
            make_identity(nc, ident)

            wsbs = []
            for name, dram in (("wq", wq_d), ("wk", wk_d), ("wv", wv_d)):
                w_sb = w_pool.tile([128, DCH * CW], f32r, tag=name)
                nc.sync.dma_start(
                    out=w_sb.rearrange("p (c w) -> p c w", c=DCH),
                    in_=dram.rearrange("(c p) w -> p c w", p=128),
                )
                wsbs.append(w_sb)
            bsbs = []
            for name, dram in (("bq", bq_d), ("bk", bk_d), ("bv", bv_d)):
                b_sb = cst_pool.tile([128, 1], f32, tag=name)
                nc.sync.dma_start(out=b_sb, in_=dram)
                bsbs.append(b_sb)
            mask_sb = cst_pool.tile([128, B * KCH], f32, tag="mask")
            nc.sync.dma_start(out=mask_sb, in_=mask_d)

            qt = qkv_pool.tile([128, BS], bf16, tag="qt")
            kt = qkv_pool.tile([128, BS], bf16, tag="kt")
            vt = qkv_pool.tile([128, BS], f32, tag="vt")
            qkv_sb = [qt, kt, vt]

            v2_tiles = {}

            def emit_proj_xts(s_):
                xts = []
                for d in range(DCH):
                    xt_t = xt_pool.tile([128, 512], f32r, tag="xt", name=f"xt{s_}_{d}")
                    nc.sync.dma_start(
                        out=xt_t,
                        in_=xT_d[d * 128 : (d + 1) * 128, s_ * 512 : (s_ + 1) * 512],
                    )
                    xts.append(xt_t)
                return xts

            def emit_proj_piece(s_, pi, xts):
                acc = ps_acc.tile([128, 512], f32, tag="acc", name=f"pj{s_}_{pi}")
                w_sb = wsbs[pi]
                for d in range(DCH):
                    nc.tensor.matmul(
                        acc,
                        w_sb[:, d * CW : (d + 1) * CW],
                        xts[d],
                        start=(d == 0),
                        stop=(d == DCH - 1),
                    )
                dst = qkv_sb[pi][:, s_ * 512 : (s_ + 1) * 512]
                nc.vector.tensor_scalar_add(dst, acc, bsbs[pi])

            def emit_proj_sblock(s_):
                xts = emit_proj_xts(s_)
                for pi in range(3):
                    emit_proj_piece(s_, pi, xts)

            def emit_v2_chunk(b, i):
                # One packed transpose: VT[128(2 heads x 64w), 128t] -> [128t, 128w].
                if (b, 0) not in v2_tiles:
                    v2 = v2_pool.tile([128, KCH * VCOLS], bf16, tag="v2", name=f"v2_{b}")
                    v2_tiles[(b, 0)] = v2
                v2 = v2_tiles[(b, 0)]
                vtr_full = ps_acc.tile([128, 512], f32, tag="acc", name=f"vtr{b}_{i}")
                vtr = vtr_full[:, 0:128]
                nc.tensor.transpose(
                    vtr, vt[:, b * S + i * 128 : b * S + (i + 1) * 128], ident
                )
                mcol = mask_sb[:, b * KCH + i : b * KCH + i + 1]
                ch = v2[:, i * VCOLS : (i + 1) * VCOLS]
                ch2 = ch.rearrange("p (g w) -> p g w", g=2)
                vtr2 = vtr.rearrange("p (g w) -> p g w", g=2)
                nc.vector.tensor_scalar_mul(ch2[:, :, 0:WH], vtr2, mcol)
                nc.vector.tensor_copy(ch[:, WH : WH + 1], mcol)
                nc.vector.tensor_copy(ch[:, VCOLS - 1 : VCOLS], mcol)

            def emit_attention_qb(b, qb, extra=()):
                # `extra`: list of (kg_slot, fn) emitted at the top of that kg
                # iteration — used to interleave next-batch proj/v2 PE work so
                # PSUM-drain latencies hide under attention matmuls.
                extra_by_kg = {}
                for slot, fn in extra:
                    extra_by_kg.setdefault(slot, []).append(fn)
                v2 = v2_tiles[(b, 0)]
                base = b * S
                qs = base + qb * 512
                ht0l = ps_ht.tile([65, 512], f32, tag="ht", name=f"ht0l_{b}_{qb}")
                ht0h = ps_ht.tile([65, 512], f32, tag="ht", name=f"ht0h_{b}_{qb}")
                ht1l = ps_ht.tile([65, 512], f32, tag="ht", name=f"ht1l_{b}_{qb}")
                ht1h = ps_ht.tile([65, 512], f32, tag="ht", name=f"ht1h_{b}_{qb}")
                for kg in range(KCH // 2):
                    for fn in extra_by_kg.get(kg, ()):
                        fn()
                    scs = []
                    for j in range(2):
                        kc = kg * 2 + j
                        sc = ps_sc.tile(
                            [128, 1024], f32, tag="sc", name=f"sc{b}_{qb}_{kc}"
                        )
                        ks = base + kc * 128
                        nc.tensor.matmul(
                            sc[:, 0:512],
                            kt[0:64, ks : ks + 128],
                            qt[0:64, qs : qs + 512],
                            start=True,
                            stop=True,
                        )
                        nc.tensor.matmul(
                            sc[:, 512:1024],
                            kt[64:128, ks : ks + 128],
                            qt[64:128, qs : qs + 512],
                            start=True,
                            stop=True,
                        )
                        scs.append(sc)
                    pbs = []
                    for j in range(2):
                        kc = kg * 2 + j
                        pb = probs_pool.tile(
                            [128, 1024], bf16, tag="pb", name=f"pb{b}_{qb}_{kc}"
                        )
                        if kc in DVE_KCS:
                            nc.vector.tensor_scalar(
                                pb.bitcast(i16),
                                scs[j],
                                SCH_A,
                                SCH_B,
                                ALU.mult,
                                ALU.add,
                            )
                        else:
                            nc.scalar.activation(pb, scs[j], AF.Exp, scale=0.125)
                        pbs.append(pb)
                    for j in range(2):
                        kc = kg * 2 + j
                        c0 = kc * VCOLS
                        # Row-split PV: contraction 128 -> two K=64 halves on
                        # alternating PE row groups (concurrent), separate
                        # PSUM banks, merged after the kc loop.
                        for (htl, hth, cs, ps) in (
                            (ht0l, ht0h, c0, slice(0, 512)),
                            (ht1l, ht1h, c0 + WH + 1, slice(512, 1024)),
                        ):
                            nc.tensor.matmul(
                                htl,
                                v2[0:64, cs : cs + WH + 1],
                                pbs[j][0:64, ps],
                                start=(kc == 0),
                                stop=(kc == KCH - 1),
                                skip_group_check=True,
                            )
                            nc.tensor.matmul(
                                hth,
                                v2[64:128, cs : cs + WH + 1],
                                pbs[j][64:128, ps],
                                start=(kc == 0),
                                stop=(kc == KCH - 1),
                                skip_group_check=True,
                            )
                for hh, htl, hth in ((0, ht0l, ht0h), (1, ht1l, ht1h)):
                    hp = hh * WH
                    hts = hts_pool.tile(
                        [65, 512], f32, tag="hts", name=f"hts{b}_{qb}_{hh}"
                    )
                    nc.scalar.copy(hts, htl)
                    nc.vector.tensor_tensor(hts, hts, hth, ALU.add)
                    ho = ho_pool.tile([128, 256], f32, tag="ho", name=f"ho{b}_{qb}_{hh}")
                    trt = ps_tr.tile([128, 144], f32, tag="tr", name=f"tr{b}_{qb}_{hh}")
                    for t in range(4):
                        tr2 = trt[:, (t % 2) * 72 : (t % 2) * 72 + 72]
                        nc.tensor.transpose(
                            tr2[:, 0:65],
                            hts[:, t * 128 : (t + 1) * 128],
                            ident[0:65, 0:65],
                        )
                        rc = rc_pool.tile(
                            [128, 1], f32, tag="rc", name=f"rc{b}_{qb}_{hh}_{t}"
                        )
                        nc.vector.reciprocal(rc, tr2[:, 64:65])
                        nc.vector.tensor_scalar_mul(
                            ho[:, t * 64 : (t + 1) * 64], tr2[:, 0:64], rc
                        )
                    dst = out_d[qs : qs + 512, hp : hp + 64].rearrange(
                        "(t p) w -> p t w", p=128
                    )
                    nc.gpsimd.dma_start(
                        out=dst, in_=ho.rearrange("p (t w) -> p t w", t=4)
                    )

            # ---- emission: proj/v2 of batch b+1 interleaved with attention(b) ----
            for s in range(4):
                emit_proj_sblock(s)
                for c in range(4 * s, 4 * s + 4):
                    emit_v2_chunk(0, c)
            for b in range(B):
                for qb in range(QB):
                    if b + 1 < B:
                        s_ = 4 * (b + 1) + qb
                        emit_proj_sblock(s_)
                        for c in range(4 * qb, 4 * qb + 4):
                            emit_v2_chunk(b + 1, c)
                    emit_attention_qb(b, qb)

    nc.compile()
    _CACHE["nc"] = nc
    return nc


def make_in_maps(x, mask, Wq, bq, Wk, bk, Wv, bv):
    x = np.asarray(x, dtype=np.float32)
    xT = np.ascontiguousarray(x.reshape(BS, D).T)
    maskT = np.ascontiguousarray(
        np.asarray(mask, dtype=np.float32)
        .reshape(B, KCH, 128)
        .transpose(2, 0, 1)
        .reshape(128, B * KCH)
    )
    in_maps = []
    for c in range(NC):
        cols = slice(c * CW, (c + 1) * CW)
        in_maps.append(
            {
                "xT": xT,
                "wqT": np.ascontiguousarray(np.asarray(Wq, np.float32)[cols, :].T),
                "wkT": np.ascontiguousarray(np.asarray(Wk, np.float32)[cols, :].T),
                "wvT": np.ascontiguousarray(np.asarray(Wv, np.float32)[cols, :].T),
                "bq": np.ascontiguousarray(np.asarray(bq, np.float32)[cols, None]),
                "bk": np.ascontiguousarray(np.asarray(bk, np.float32)[cols, None]),
                "bv": np.ascontiguousarray(np.asarray(bv, np.float32)[cols, None]),
                "maskT": maskT,
            }
        )
    return in_maps


def assemble(results):
    out = np.empty((BS, D), dtype=np.float32)
    for c in range(NC):
        out[:, c * CW : (c + 1) * CW] = results[c]["h_out"]
    return out.reshape(B, S, D)


def kernel(x, mask, Wq, bq, Wk, bk, Wv, bv, **run_kwargs):
    _ensure_import()
    from concourse.bass_utils import run_bass_kernel_spmd

    nc = build_bass()
    in_maps = make_in_maps(x, mask, Wq, bq, Wk, bk, Wv, bv)
    res = run_bass_kernel_spmd(nc, in_maps, core_ids=list(range(NC)), **run_kwargs)
    _CACHE["last_results"] = res
    return assemble(res.results)


# revision 35
# speedup vs baseline: 1.6408x; 1.6408x over previous
# Multi-headed self-attention (B=4, S=2048, D=1024, H=16) on 8 TRN2 NeuronCores.
#
# Sharding: tensor-parallel over heads. Core c computes heads 2c, 2c+1 (=128
# output columns) for all batches. Host pre-transposes x -> xT [D, B*S] and the
# per-core weight slices -> [D, 128] so every matmul contracts over the
# partition dimension. Host gathers the 8 [B*S, 128] outputs into (B,S,D).
#
# Per-core dataflow (fp32 storage, float32r matmul mode):
#   1. Projections: QT/KT/VT [128(2 heads x 64), 8192] = W.T-slices @ xT,
#      accumulated over 8 d-chunks in PSUM; bias added during the PSUM->SBUF
#      copy (DVE per-partition scalar add).
#   2. V2 prep: one packed PE transpose per 128-t chunk turns VT[128(2 heads),
#      128t] into [128t, 128w]; DVE applies the key mask and appends a mask
#      column per head -> v2 chunk layout [V_h0(64)|m|V_h1(64)|m] (130 cols).
#   3. Attention per (batch, q-block): scoresT tiles [128 k, 512 q] for BOTH
#      heads per k-chunk, head0 on PE row-tile (0,0) (partitions 0:64), head1
#      on row-tile (64,0) -> the two K=64 matmuls execute concurrently on
#      disjoint row groups. exp on alternating engines: ScalarE exact Exp
#      (fused 1/8 scale) for even k-chunks, VectorE Schraudolph bit-trick exp
#      (one tensor_scalar: int32(x*A+B) reinterpreted as f32, ~1.8% rel err,
#      softmax-safe) for odd k-chunks. No row-max subtraction (scores std
#      ~0.4, exp is safe in fp32, softmax is shift-invariant). PV matmuls use
#      the 65-col stationary [V|mask] so the accumulation yields unnormalized
#      h^T plus the softmax denominator. PE-transpose h''^T back to [q, 65],
#      DVE reciprocal of column 64, per-partition scalar multiply, DMA out.
#   The 0/1 mask is exact this way: reference's exp(-10000) == 0.0 in fp32.
#   Emission interleaves proj/v2-prep of batch b+1 with attention of batch b
#   to keep the PE dense; hts PSUM->SBUF copies run on GpSimd.

import sys

import numpy as np

B, S, D, H = 4, 2048, 1024, 16
NC = 8
HPC = H // NC  # heads per core = 2
WH = D // H  # head width = 64
CW = HPC * WH  # per-core output width = 128
BS = B * S  # 8192
DCH = D // 128  # d chunks = 8
QB = S // 512  # q blocks per batch = 4
KCH = S // 128  # k chunks per batch = 16
VCOLS = 2 * (WH + 1)  # v2 chunk cols = 130

# Schraudolph exp in bf16: exp(x*0.125) ~= bitcast_bf16(int16(x*A + Bc))
# (bf16 = 8-bit exp, 7-bit mantissa -> the int domain is 2^7 per octave)
_LN2 = float(np.log(2.0))
SCH_A = 0.125 * (2**7) / _LN2
SCH_B = 127.0 * (2**7) - 5.5
# k-chunks whose exp runs on VectorE (Schraudolph); rest on ScalarE (exact).
DVE_KCS = frozenset({1, 3, 5, 8, 10, 13, 15})

_CACHE = {}


def _ensure_import():
    try:
        import concourse.bass  # noqa: F401
    except ImportError:
        sys.path.insert(0, "/opt/trn_rl_repo")
        import concourse.bass  # noqa: F401


def build_bass():
    if "nc" in _CACHE:
        return _CACHE["nc"]
    _ensure_import()
    import concourse.mybir as mybir
    import concourse.tile as tile
    from concourse import bacc
    from concourse.masks import make_identity

    f32 = mybir.dt.float32
    f32r = mybir.dt.float32r
    bf16 = mybir.dt.bfloat16
    i16 = mybir.dt.int16
    AF = mybir.ActivationFunctionType
    ALU = mybir.AluOpType

    nc = bacc.Bacc(
        "TRN2",
        target_bir_lowering=False,
        debug=False,
        enable_asserts=False,
        num_devices=NC,
    )
    xT_d = nc.dram_tensor("xT", (D, BS), bf16, kind="ExternalInput").ap()
    wq_d = nc.dram_tensor("wqT", (D, CW), bf16, kind="ExternalInput").ap()
    wk_d = nc.dram_tensor("wkT", (D, CW), bf16, kind="ExternalInput").ap()
    wv_d = nc.dram_tensor("wvT", (D, CW), bf16, kind="ExternalInput").ap()
    bq_d = nc.dram_tensor("bq", (CW, 1), f32, kind="ExternalInput").ap()
    bk_d = nc.dram_tensor("bk", (CW, 1), f32, kind="ExternalInput").ap()
    bv_d = nc.dram_tensor("bv", (CW, 1), f32, kind="ExternalInput").ap()
    mask_d = nc.dram_tensor("maskT", (128, B * KCH), f32, kind="ExternalInput").ap()
    out_d = nc.dram_tensor("h_out", (BS, CW), f32, kind="ExternalOutput").ap()

    with tile.TileContext(nc) as tc:
        with (
            tc.tile_pool(name="qkv", bufs=1) as qkv_pool,
            tc.tile_pool(name="xt", bufs=20) as xt_pool,
            tc.tile_pool(name="wsb", bufs=1) as w_pool,
            tc.tile_pool(name="probs", bufs=4) as probs_pool,
            tc.tile_pool(name="v2", bufs=2) as v2_pool,
            tc.tile_pool(name="hts", bufs=4) as hts_pool,
            tc.tile_pool(name="ho", bufs=3) as ho_pool,
            tc.tile_pool(name="rc", bufs=8) as rc_pool,
            tc.tile_pool(name="cst", bufs=1) as cst_pool,
            tc.tile_pool(name="ps_sc", bufs=2, space="PSUM") as ps_sc,
            tc.tile_pool(name="ps_ht", bufs=2, space="PSUM") as ps_ht,
            tc.tile_pool(name="ps_acc", bufs=1, space="PSUM") as ps_acc,
            tc.tile_pool(name="ps_tr", bufs=1, space="PSUM") as ps_tr,
        ):
            ident = cst_pool.tile([128, 128], f32, tag="ident")
            make_identity(nc, ident)

            wsbs = []
            for name, dram in (("wq", wq_d), ("wk", wk_d), ("wv", wv_d)):
                w_sb = w_pool.tile([128, DCH * CW], bf16, tag=name)
                nc.sync.dma_start(
                    out=w_sb.rearrange("p (c w) -> p c w", c=DCH),
                    in_=dram.rearrange("(c p) w -> p c w", p=128),
                )
                wsbs.append(w_sb)
            bsbs = []
            for name, dram in (("bq", bq_d), ("bk", bk_d), ("bv", bv_d)):
                b_sb = cst_pool.tile([128, 1], f32, tag=name)
                nc.sync.dma_start(out=b_sb, in_=dram)
                bsbs.append(b_sb)
            mask_sb = cst_pool.tile([128, B * KCH], f32, tag="mask")
            nc.sync.dma_start(out=mask_sb, in_=mask_d)

            qt = qkv_pool.tile([128, BS], bf16, tag="qt")
            kt = qkv_pool.tile([128, BS], bf16, tag="kt")
            vt = qkv_pool.tile([128, BS], f32, tag="vt")
            qkv_sb = [qt, kt, vt]

            v2_tiles = {}

            def emit_proj_xts(s_):
                xts = []
                for d in range(DCH):
                    xt_t = xt_pool.tile([128, 512], bf16, tag="xt", name=f"xt{s_}_{d}")
                    nc.sync.dma_start(
                        out=xt_t,
                        in_=xT_d[d * 128 : (d + 1) * 128, s_ * 512 : (s_ + 1) * 512],
                    )
                    xts.append(xt_t)
                return xts

            def emit_proj_piece(s_, pi, xts):
                acc = ps_acc.tile([128, 512], f32, tag="acc", name=f"pj{s_}_{pi}")
                w_sb = wsbs[pi]
                for d in range(DCH):
                    nc.tensor.matmul(
                        acc,
                        w_sb[:, d * CW : (d + 1) * CW],
                        xts[d],
                        start=(d == 0),
                        stop=(d == DCH - 1),
                    )
                dst = qkv_sb[pi][:, s_ * 512 : (s_ + 1) * 512]
                nc.vector.tensor_scalar_add(dst, acc, bsbs[pi])

            def emit_proj_sblock(s_):
                xts = emit_proj_xts(s_)
                for pi in range(3):
                    emit_proj_piece(s_, pi, xts)

            def emit_v2_chunk(b, i):
                # One packed transpose: VT[128(2 heads x 64w), 128t] -> [128t, 128w].
                if (b, 0) not in v2_tiles:
                    v2 = v2_pool.tile([128, KCH * VCOLS], bf16, tag="v2", name=f"v2_{b}")
                    v2_tiles[(b, 0)] = v2
                v2 = v2_tiles[(b, 0)]
                vtr_full = ps_acc.tile([128, 512], f32, tag="acc", name=f"vtr{b}_{i}")
                vtr = vtr_full[:, 0:128]
                nc.tensor.transpose(
                    vtr, vt[:, b * S + i * 128 : b * S + (i + 1) * 128], ident
                )
                mcol = mask_sb[:, b * KCH + i : b * KCH + i + 1]
                ch = v2[:, i * VCOLS : (i + 1) * VCOLS]
                ch2 = ch.rearrange("p (g w) -> p g w", g=2)
                vtr2 = vtr.rearrange("p (g w) -> p g w", g=2)
                nc.vector.tensor_scalar_mul(ch2[:, :, 0:WH], vtr2, mcol)
                nc.vector.tensor_copy(ch[:, WH : WH + 1], mcol)
                nc.vector.tensor_copy(ch[:, VCOLS - 1 : VCOLS], mcol)

            def emit_attention_qb(b, qb, extra=()):
                # `extra`: list of (kg_slot, fn) emitted at the top of that kg
                # iteration — used to interleave next-batch proj/v2 PE work so
                # PSUM-drain latencies hide under attention matmuls.
                extra_by_kg = {}
                for slot, fn in extra:
                    extra_by_kg.setdefault(slot, []).append(fn)
                v2 = v2_tiles[(b, 0)]
                base = b * S
                qs = base + qb * 512
                ht0 = ps_ht.tile([65, 512], f32, tag="ht", name=f"ht0_{b}_{qb}")
                ht1 = ps_ht.tile([65, 512], f32, tag="ht", name=f"ht1_{b}_{qb}")
                for kg in range(KCH // 2):
                    for fn in extra_by_kg.get(kg, ()):
                        fn()
                    scs = []
                    for j in range(2):
                        kc = kg * 2 + j
                        sc = ps_sc.tile(
                            [128, 1024], f32, tag="sc", name=f"sc{b}_{qb}_{kc}"
                        )
                        ks = base + kc * 128
                        nc.tensor.matmul(
                            sc[:, 0:512],
                            kt[0:64, ks : ks + 128],
                            qt[0:64, qs : qs + 512],
                            start=True,
                            stop=True,
                        )
                        nc.tensor.matmul(
                            sc[:, 512:1024],
                            kt[64:128, ks : ks + 128],
                            qt[64:128, qs : qs + 512],
                            start=True,
                            stop=True,
                        )
                        scs.append(sc)
                    pbs = []
                    for j in range(2):
                        kc = kg * 2 + j
                        pb = probs_pool.tile(
                            [128, 1024], bf16, tag="pb", name=f"pb{b}_{qb}_{kc}"
                        )
                        if kc in DVE_KCS:
                            nc.vector.tensor_scalar(
                                pb.bitcast(i16),
                                scs[j],
                                SCH_A,
                                SCH_B,
                                ALU.mult,
                                ALU.add,
                            )
                        else:
                            nc.scalar.activation(pb, scs[j], AF.Exp, scale=0.125)
                        pbs.append(pb)
                    for j in range(2):
                        kc = kg * 2 + j
                        c0 = kc * VCOLS
                        nc.tensor.matmul(
                            ht0,
                            v2[:, c0 : c0 + WH + 1],
                            pbs[j][:, 0:512],
                            start=(kc == 0),
                            stop=(kc == KCH - 1),
                            skip_group_check=True,
                        )
                        nc.tensor.matmul(
                            ht1,
                            v2[:, c0 + WH + 1 : c0 + VCOLS],
                            pbs[j][:, 512:1024],
                            start=(kc == 0),
                            stop=(kc == KCH - 1),
                            skip_group_check=True,
                        )
                for hh, ht in ((0, ht0), (1, ht1)):
                    hp = hh * WH
                    hts = hts_pool.tile(
                        [65, 512], f32, tag="hts", name=f"hts{b}_{qb}_{hh}"
                    )
                    if hh == 0:
                        nc.scalar.copy(hts, ht)
                    else:
                        nc.vector.tensor_copy(hts, ht)
                    ho = ho_pool.tile([128, 256], f32, tag="ho", name=f"ho{b}_{qb}_{hh}")
                    trt = ps_tr.tile([128, 144], f32, tag="tr", name=f"tr{b}_{qb}_{hh}")
                    for t in range(4):
                        tr2 = trt[:, (t % 2) * 72 : (t % 2) * 72 + 72]
                        nc.tensor.transpose(
                            tr2[:, 0:65],
                            hts[:, t * 128 : (t + 1) * 128],
                            ident[0:65, 0:65],
                        )
                        rc = rc_pool.tile(
                            [128, 1], f32, tag="rc", name=f"rc{b}_{qb}_{hh}_{t}"
                        )
                        nc.vector.reciprocal(rc, tr2[:, 64:65])
                        nc.vector.tensor_scalar_mul(
                            ho[:, t * 64 : (t + 1) * 64], tr2[:, 0:64], rc
                        )
                    dst = out_d[qs : qs + 512, hp : hp + 64].rearrange(
                        "(t p) w -> p t w", p=128
                    )
                    nc.gpsimd.dma_start(
                        out=dst, in_=ho.rearrange("p (t w) -> p t w", t=4)
                    )

            # ---- emission: proj/v2 of batch b+1 interleaved with attention(b) ----
            for s in range(4):
                emit_proj_sblock(s)
                for c in range(4 * s, 4 * s + 4):
                    emit_v2_chunk(0, c)
            for b in range(B):
                for qb in range(QB):
                    if b + 1 < B:
                        s_ = 4 * (b + 1) + qb
                        emit_proj_sblock(s_)
                        for c in range(4 * qb, 4 * qb + 4):
                            emit_v2_chunk(b + 1, c)
                    emit_attention_qb(b, qb)

    nc.compile()
    _CACHE["nc"] = nc
    return nc


def make_in_maps(x, mask, Wq, bq, Wk, bk, Wv, bv):
    import ml_dtypes

    bf = ml_dtypes.bfloat16
    x = np.asarray(x, dtype=np.float32)
    xT = np.ascontiguousarray(x.reshape(BS, D).T.astype(bf))
    maskT = np.ascontiguousarray(
        np.asarray(mask, dtype=np.float32)
        .reshape(B, KCH, 128)
        .transpose(2, 0, 1)
        .reshape(128, B * KCH)
    )
    in_maps = []
    for c in range(NC):
        cols = slice(c * CW, (c + 1) * CW)
        in_maps.append(
            {
                "xT": xT,
                "wqT": np.ascontiguousarray(np.asarray(Wq, np.float32)[cols, :].T.astype(bf)),
                "wkT": np.ascontiguousarray(np.asarray(Wk, np.float32)[cols, :].T.astype(bf)),
                "wvT": np.ascontiguousarray(np.asarray(Wv, np.float32)[cols, :].T.astype(bf)),
                "bq": np.ascontiguousarray(np.asarray(bq, np.float32)[cols, None]),
                "bk": np.ascontiguousarray(np.asarray(bk, np.float32)[cols, None]),
                "bv": np.ascontiguousarray(np.asarray(bv, np.float32)[cols, None]),
                "maskT": maskT,
            }
        )
    return in_maps


def assemble(results):
    out = np.empty((BS, D), dtype=np.float32)
    for c in range(NC):
        out[:, c * CW : (c + 1) * CW] = results[c]["h_out"]
    return out.reshape(B, S, D)


def kernel(x, mask, Wq, bq, Wk, bk, Wv, bv, **run_kwargs):
    _ensure_import()
    from concourse.bass_utils import run_bass_kernel_spmd

    nc = build_bass()
    in_maps = make_in_maps(x, mask, Wq, bq, Wk, bk, Wv, bv)
    res = run_bass_kernel_spmd(nc, in_maps, core_ids=list(range(NC)), **run_kwargs)
    _CACHE["last_results"] = res
    return assemble(res.results)
